# revision 4
# baseline (speedup 1.0000x reference)
"""Trainium2 Bass kernel for GQA attention (B=2, S=2048, D=1024, H=16, HKV=4).

Sharding: 8 cores = batch(2) x kv-group(4). Each core handles one batch and
one KV head group (4 query heads + 1 KV head), computes attention plus its
partial slice of the output projection (row-parallel wo); the host sums the
4 tensor-parallel partials per batch. No device collectives needed.

Device kernel (per core, all matmul operands bf16, fp32 PSUM accumulation):
  1. QKV projection:  qkv[s,:] = x[s,:] @ [wq|wk|wv]^T   (xT chunks stationary)
  2. RoPE on q,k in natural layout. Weights rows are pre-permuted on host so
     each head's dims are [32 real | 32 imag] making all rope slices
     contiguous (no stride-2 access).
  3. PE-transpose roped q,k -> qT [hd, s], kT (kT duplicated to partitions
     64-127 so two heads run concurrently via PE row tiling).
  4. Scores transposed: S^T[k,q] = kT.T @ qT per 128-row k-block, causal
     upper blocks skipped entirely. exp on ACT with scale=1/8 folded in (no
     max subtraction: |scores| <~ 5 for this data, masked entries skipped).
     Diagonal 128x128 block masked by a binary multiply after exp.
  5. PV with a ones-row appended to V: out^T[d,q] accumulates over k-blocks
     in PSUM; row 64 collects the softmax denominator for free.
  6. Normalize with reciprocal + partition-broadcast multiply, then
     y_partial[s,:] = attn^T.T @ woT accumulated over the two head-pairs.
"""

import numpy as np
import ml_dtypes

B, S, D = 2, 2048, 1024
H, HKV, HD = 16, 4, 64
REP = H // HKV  # 4 query heads per kv head
N_CORES = 8
NSB = S // 128  # 16 s-blocks
NDC = D // 128  # 8 d-chunks
QKV = REP * HD + 2 * HD  # 384 projected dims per core (256 q + 64 k + 64 v)
BF16 = ml_dtypes.bfloat16

# col offsets of each kb's exp-score span inside the per-head expS buffer
OFF = [0]
for _kb in range(NSB):
    OFF.append(OFF[-1] + (S - 128 * _kb))
TOT = OFF[-1]  # 17408

_CACHE = {}


def _build_module():
    from contextlib import ExitStack

    import concourse.bacc as bacc
    import concourse.mybir as mybir
    import concourse.tile as tile
    from concourse.alu_op_type import AluOpType

    f32 = mybir.dt.float32
    bf16 = mybir.dt.bfloat16
    Exp = mybir.ActivationFunctionType.Exp

    nc = bacc.Bacc("TRN2", target_bir_lowering=False, debug=False,
                   num_devices=N_CORES)

    xT_d = nc.dram_tensor("xT", (D, S), bf16, kind="ExternalInput").ap()
    wcat_d = nc.dram_tensor("wcatT", (D, QKV), bf16, kind="ExternalInput").ap()
    woT_d = nc.dram_tensor("woT", (2 * 128, D), bf16, kind="ExternalInput").ap()
    ctq_d = nc.dram_tensor("ctq", (128, S), bf16, kind="ExternalInput").ap()
    stq_d = nc.dram_tensor("stq", (128, S), bf16, kind="ExternalInput").ap()
    ctk_d = nc.dram_tensor("ctk", (128, NSB * 32), bf16, kind="ExternalInput").ap()
    stk_d = nc.dram_tensor("stk", (128, NSB * 32), bf16, kind="ExternalInput").ap()
    mb_d = nc.dram_tensor("maskb", (128, 128), bf16, kind="ExternalInput").ap()
    idn_d = nc.dram_tensor("ident", (128, 128), bf16, kind="ExternalInput").ap()
    y_d = nc.dram_tensor("y", (S, D), f32, kind="ExternalOutput").ap()

    with tile.TileContext(nc) as tc:
        with ExitStack() as ctx:
            persist = ctx.enter_context(tc.tile_pool(name="persist", bufs=1))
            xT = persist.tile([128, NDC * S], bf16)      # 32 KB/part
            wcat = persist.tile([128, NDC * QKV], bf16)  # 6 KB
            woT = persist.tile([128, 2 * D], bf16)       # 4 KB
            ctq = persist.tile([128, S], bf16)
            stq = persist.tile([128, S], bf16)
            ctk = persist.tile([128, NSB * 32], bf16)
            stk = persist.tile([128, NSB * 32], bf16)
            mb = persist.tile([128, 128], bf16)
            idn = persist.tile([128, 128], bf16)
            qT = persist.tile([128, 2 * S], bf16)        # 2 head-pair blocks
            kT2 = persist.tile([128, S], bf16)           # kT stacked twice
            vb = persist.tile([128, NSB * 65], bf16)     # v + ones column
            attnT0 = persist.tile([128, S], bf16, tag="attnT0")
            attnT1 = persist.tile([128, S], bf16, tag="attnT1")
            attnT = [attnT0, attnT1]

            # input DMAs
            for dc in range(NDC):
                nc.sync.dma_start(xT[:, dc * S:(dc + 1) * S],
                                  xT_d[dc * 128:(dc + 1) * 128, :])
                nc.sync.dma_start(wcat[:, dc * QKV:(dc + 1) * QKV],
                                  wcat_d[dc * 128:(dc + 1) * 128, :])
            for c in range(2):
                nc.sync.dma_start(woT[:, c * D:(c + 1) * D],
                                  woT_d[c * 128:(c + 1) * 128, :])
            nc.sync.dma_start(ctq[:], ctq_d[:])
            nc.sync.dma_start(stq[:], stq_d[:])
            nc.sync.dma_start(ctk[:], ctk_d[:])
            nc.sync.dma_start(stk[:], stk_d[:])
            nc.sync.dma_start(mb[:], mb_d[:])
            nc.sync.dma_start(idn[:], idn_d[:])
            nc.gpsimd.memset(vb[:], 1.0)

            # ---- stage 1: qkv projection + rope + transposes ----
            with ExitStack() as s1:
                psq = s1.enter_context(
                    tc.tile_pool(name="psqkv", bufs=2, space="PSUM"))
                tps = s1.enter_context(
                    tc.tile_pool(name="tpsum", bufs=3, space="PSUM"))
                qkp = s1.enter_context(tc.tile_pool(name="qknat", bufs=3))
                qRp = s1.enter_context(tc.tile_pool(name="qkrope", bufs=3))
                tmp = s1.enter_context(tc.tile_pool(name="ropetmp", bufs=2))

                for sb in range(NSB):
                    ps = psq.tile([128, QKV], f32)
                    for dc in range(NDC):
                        nc.tensor.matmul(
                            ps[:],
                            lhsT=xT[:, dc * S + sb * 128: dc * S + (sb + 1) * 128],
                            rhs=wcat[:, dc * QKV:(dc + 1) * QKV],
                            start=(dc == 0), stop=(dc == NDC - 1))
                    qk = qkp.tile([128, 320], bf16)
                    nc.vector.tensor_copy(qk[:], ps[:, 0:320])
                    nc.vector.tensor_copy(vb[:, sb * 65: sb * 65 + 64],
                                          ps[:, 320:384])

                    # rope: out_r = r*c - i*s ; out_i = r*s + i*c
                    qR = qRp.tile([128, 320], bf16)
                    qv = qk[:, 0:256].rearrange("p (h hd) -> p h hd", h=REP)
                    qRv = qR[:, 0:256].rearrange("p (h hd) -> p h hd", h=REP)
                    qr, qi = qv[:, :, 0:32], qv[:, :, 32:64]
                    qRr, qRi = qRv[:, :, 0:32], qRv[:, :, 32:64]
                    ct = ctq[:, sb * 128:(sb + 1) * 128].rearrange(
                        "p (h j) -> p h j", h=REP)
                    st = stq[:, sb * 128:(sb + 1) * 128].rearrange(
                        "p (h j) -> p h j", h=REP)
                    tA = tmp.tile([128, 128], bf16, tag="tA")
                    tB = tmp.tile([128, 128], bf16, tag="tB")
                    tAv = tA[:].rearrange("p (h j) -> p h j", h=REP)
                    tBv = tB[:].rearrange("p (h j) -> p h j", h=REP)
                    nc.vector.tensor_tensor(tAv, qr, ct, AluOpType.mult)
                    nc.vector.tensor_tensor(tBv, qi, st, AluOpType.mult)
                    nc.vector.tensor_tensor(qRr, tAv, tBv, AluOpType.subtract)
                    nc.vector.tensor_tensor(tAv, qr, st, AluOpType.mult)
                    nc.vector.tensor_tensor(tBv, qi, ct, AluOpType.mult)
                    nc.vector.tensor_tensor(qRi, tAv, tBv, AluOpType.add)

                    kr, ki = qk[:, 256:288], qk[:, 288:320]
                    kRr, kRi = qR[:, 256:288], qR[:, 288:320]
                    ctks = ctk[:, sb * 32:(sb + 1) * 32]
                    stks = stk[:, sb * 32:(sb + 1) * 32]
                    tC = tmp.tile([128, 32], bf16, tag="tC")
                    tD = tmp.tile([128, 32], bf16, tag="tD")
                    nc.vector.tensor_tensor(tC[:], kr, ctks, AluOpType.mult)
                    nc.vector.tensor_tensor(tD[:], ki, stks, AluOpType.mult)
                    nc.vector.tensor_tensor(kRr, tC[:], tD[:], AluOpType.subtract)
                    nc.vector.tensor_tensor(tC[:], kr, stks, AluOpType.mult)
                    nc.vector.tensor_tensor(tD[:], ki, ctks, AluOpType.mult)
                    nc.vector.tensor_tensor(kRi, tC[:], tD[:], AluOpType.add)

                    # transposes to qT / kT2
                    for hb in range(2):
                        pt = tps.tile([128, 128], bf16, tag="ptq")
                        nc.tensor.transpose(pt[:], qR[:, hb * 128:(hb + 1) * 128],
                                            idn[:])
                        nc.vector.tensor_copy(
                            qT[:, hb * S + sb * 128: hb * S + (sb + 1) * 128],
                            pt[:])
                    ptk = tps.tile([64, 128], bf16, tag="ptk")
                    nc.tensor.transpose(ptk[:], qR[:, 256:320], idn[:])
                    nc.vector.tensor_copy(kT2[0:64, sb * 128:(sb + 1) * 128],
                                          ptk[:])

            # replicate kT to partitions 64-127 (cross-partition: use DMA)
            nc.sync.dma_start(kT2[64:128, :], kT2[0:64, :])

            # ---- stage 2: attention per head-pair ----
            with ExitStack() as s2:
                psS = s2.enter_context(
                    tc.tile_pool(name="psS", bufs=2, space="PSUM"))
                psO = s2.enter_context(
                    tc.tile_pool(name="psO", bufs=1, space="PSUM"))
                eSp = s2.enter_context(tc.tile_pool(name="expS", bufs=1))
                recp = s2.enter_context(tc.tile_pool(name="recip", bufs=2))
                otp = s2.enter_context(tc.tile_pool(name="oddtmp", bufs=1))

                for hp in range(2):
                    eS = [eSp.tile([128, TOT], bf16, tag=f"eS{i}",
                                   name=f"eS{hp}_{i}")
                          for i in range(2)]
                    for kb in range(NSB):
                        for qc in range(kb // 4, 4):
                            gs = max(qc * 512, kb * 128)
                            ge = (qc + 1) * 512
                            n = ge - gs
                            for i in range(2):
                                pst = psS.tile([128, 512], f32, tag=f"psS{i}")
                                nc.tensor.matmul(
                                    pst[:, 0:n],
                                    lhsT=kT2[i * 64:(i + 1) * 64,
                                             kb * 128:(kb + 1) * 128],
                                    rhs=qT[i * 64:(i + 1) * 64,
                                           hp * S + gs: hp * S + ge],
                                    start=True, stop=True)
                                nc.scalar.activation(
                                    eS[i][:, OFF[kb] + gs - kb * 128:
                                          OFF[kb] + ge - kb * 128],
                                    pst[:, 0:n], Exp, scale=0.125)
                        # mask the diagonal 128x128 block (strictly-lower = 0)
                        for i in range(2):
                            dsl = eS[i][:, OFF[kb]:OFF[kb] + 128]
                            nc.vector.tensor_tensor(dsl, dsl, mb[:],
                                                    AluOpType.mult)
                    for i in range(2):
                        po = psO.tile([65, S], f32)
                        for kb in range(NSB):
                            for qc in range(kb // 4, 4):
                                gs = max(qc * 512, kb * 128)
                                ge = (qc + 1) * 512
                                nc.tensor.matmul(
                                    po[:, gs:ge],
                                    lhsT=vb[:, kb * 65: kb * 65 + 65],
                                    rhs=eS[i][:, OFF[kb] + gs - kb * 128:
                                              OFF[kb] + ge - kb * 128],
                                    start=(kb == 0),
                                    stop=(kb == min(NSB - 1, 4 * qc + 3)))
                        rc = recp.tile([1, S], f32)
                        nc.vector.reciprocal(rc[:], po[64:65, :])
                        rcb = recp.tile([64, S], f32, tag="rcb")
                        nc.gpsimd.partition_broadcast(rcb[:], rc[0:1, :])
                        if i == 0:
                            nc.vector.tensor_tensor(attnT[hp][0:64, :],
                                                    po[0:64, :], rcb[:],
                                                    AluOpType.mult)
                        else:
                            ot = otp.tile([64, S], bf16)
                            nc.vector.tensor_tensor(ot[:], po[0:64, :], rcb[:],
                                                    AluOpType.mult)
                            nc.sync.dma_start(attnT[hp][64:128, :], ot[:])

            # ---- stage 3: output projection ----
            with ExitStack() as s3:
                psY = s3.enter_context(
                    tc.tile_pool(name="psY", bufs=2, space="PSUM"))
                yst = s3.enter_context(tc.tile_pool(name="ystage", bufs=3))
                for sb in range(NSB):
                    yp = psY.tile([128, D], f32)
                    for hp in range(2):
                        for c2 in range(2):
                            nc.tensor.matmul(
                                yp[:, c2 * 512:(c2 + 1) * 512],
                                lhsT=attnT[hp][:, sb * 128:(sb + 1) * 128],
                                rhs=woT[:, hp * D + c2 * 512:
                                        hp * D + (c2 + 1) * 512],
                                start=(hp == 0), stop=(hp == 1))
                    ys = yst.tile([128, D], f32)
                    if sb % 2 == 0:
                        nc.scalar.copy(ys[:], yp[:])
                    else:
                        nc.vector.tensor_copy(ys[:], yp[:])
                    nc.sync.dma_start(y_d[sb * 128:(sb + 1) * 128, :], ys[:])

    nc.compile()
    return nc


def _get_module():
    if "nc" not in _CACHE:
        _CACHE["nc"] = _build_module()
    return _CACHE["nc"]


def _host_tables(freqs_cos, freqs_sin):
    # ctq[p, sb*128 + h*32 + j] = cos[sb*128+p, j]  (tiled over 4 heads)
    c3 = freqs_cos.reshape(NSB, 128, 32)
    s3_ = freqs_sin.reshape(NSB, 128, 32)
    ctq = np.tile(c3, (1, 1, REP)).transpose(1, 0, 2).reshape(128, S)
    stq = np.tile(s3_, (1, 1, REP)).transpose(1, 0, 2).reshape(128, S)
    ctk = c3.transpose(1, 0, 2).reshape(128, NSB * 32)
    stk = s3_.transpose(1, 0, 2).reshape(128, NSB * 32)
    return ctq, stq, ctk, stk


def make_in_maps(x, wq, wk, wv, wo, freqs_cos, freqs_sin):
    x = np.asarray(x, np.float32)
    wq = np.asarray(wq, np.float32)
    wk = np.asarray(wk, np.float32)
    wv = np.asarray(wv, np.float32)
    wo = np.asarray(wo, np.float32)
    freqs_cos = np.asarray(freqs_cos, np.float32)
    freqs_sin = np.asarray(freqs_sin, np.float32)

    # deinterleave rope pairs within each head: [r0 i0 r1 i1 ...] ->
    # [r0..r31 | i0..i31]
    idx = np.concatenate([np.arange(0, HD, 2), np.arange(1, HD, 2)])
    wq_p = wq.reshape(H, HD, D)[:, idx, :].reshape(H * HD, D)
    wk_p = wk.reshape(HKV, HD, D)[:, idx, :].reshape(HKV * HD, D)

    ctq, stq, ctk, stk = _host_tables(freqs_cos, freqs_sin)
    maskb = (np.arange(128)[:, None] <= np.arange(128)[None, :])
    ident = np.eye(128)

    common = {
        "ctq": ctq.astype(BF16), "stq": stq.astype(BF16),
        "ctk": ctk.astype(BF16), "stk": stk.astype(BF16),
        "maskb": maskb.astype(BF16), "ident": ident.astype(BF16),
    }
    xT_b = [np.ascontiguousarray(x[b].T).astype(BF16) for b in range(B)]
    in_maps = []
    for core in range(N_CORES):
        b, g = divmod(core, HKV)
        wqT = wq_p[g * 256:(g + 1) * 256].T
        wkT = wk_p[g * 64:(g + 1) * 64].T
        wvT = wv[g * 64:(g + 1) * 64].T
        wcat = np.ascontiguousarray(
            np.concatenate([wqT, wkT, wvT], axis=1)).astype(BF16)
        woTg = np.ascontiguousarray(wo[:, g * 256:(g + 1) * 256].T).astype(BF16)
        in_maps.append({"xT": xT_b[b], "wcatT": wcat, "woT": woTg, **common})
    return in_maps


def _causal_fast_path_ok(mask):
    m = np.asarray(mask)
    if m.shape != (S, S):
        return False
    upper = m[np.triu_indices(S, 1)]
    lower = m[np.tril_indices(S, 0)]
    return bool(np.all(upper <= -1e8) and np.all(lower == 0))


def _numpy_fallback(x, wq, wk, wv, wo, freqs_cos, freqs_sin, mask):
    x = np.asarray(x, np.float32)
    xq = (x.reshape(B * S, D) @ np.asarray(wq, np.float32).T).reshape(B, S, H, HD)
    xk = (x.reshape(B * S, D) @ np.asarray(wk, np.float32).T).reshape(B, S, HKV, HD)
    xv = (x.reshape(B * S, D) @ np.asarray(wv, np.float32).T).reshape(B, S, HKV, HD)

    def rope(t, nh):
        tf = t.reshape(B, S, nh, HD // 2, 2)
        tr, ti = tf[..., 0], tf[..., 1]
        c = np.asarray(freqs_cos, np.float32)[None, :, None, :]
        s = np.asarray(freqs_sin, np.float32)[None, :, None, :]
        outr = tr * c - ti * s
        outi = tr * s + ti * c
        return np.stack([outr, outi], axis=-1).reshape(B, S, nh, HD)

    xq = rope(xq, H)
    xk = rope(xk, HKV)
    xqg = xq.reshape(B, S, HKV, REP, HD)
    scores = np.einsum("bqgrd,bkgd->bgrqk", xqg, xk) / np.sqrt(np.float32(HD))
    scores = scores + np.asarray(mask, np.float32)[None, None, None, :, :]
    scores = scores - scores.max(axis=-1, keepdims=True)
    e = np.exp(scores)
    attn = e / e.sum(axis=-1, keepdims=True)
    out = np.einsum("bgrqk,bkgd->bqgrd", attn, xv).reshape(B, S, H * HD)
    return (out.reshape(B * S, H * HD) @ np.asarray(wo, np.float32)
            .T.astype(np.float32)).reshape(B, S, D).astype(np.float32)


def kernel(x, wq, wk, wv, wo, freqs_cos, freqs_sin, mask):
    if not _causal_fast_path_ok(mask):
        return _numpy_fallback(x, wq, wk, wv, wo, freqs_cos, freqs_sin, mask)
    from concourse import bass_utils
    nc = _get_module()
    in_maps = make_in_maps(x, wq, wk, wv, wo, freqs_cos, freqs_sin)
    res = bass_utils.run_bass_kernel_spmd(nc, in_maps,
                                          core_ids=list(range(N_CORES)))
    y = np.zeros((B, S, D), np.float32)
    for core in range(N_CORES):
        b = core // HKV
        y[b] += res.results[core]["y"]
    return y
